# revision 15
# baseline (speedup 1.0000x reference)
"""Dense graph-attention layer (GAT) on 8 Trainium2 NeuronCores.

Sharding: data-parallel over batch B=8 -> one batch element per core.
Adjacency mask and per-head params are replicated.

Key identity (exact, no approximation):
  leaky_relu(t, 0.2) = 0.2*t + 0.8*relu(t)
  exp(leaky(dst_i + src_j) + p_j)
    = e^{0.2 dst_i} * e^{0.2 src_j + p_j} * max(e^{0.8(dst_i+src_j)}, 1)

  - the per-i factor e^{0.2 dst_i} cancels between softmax numerator and
    denominator -> dropped entirely.
  - the per-j factor s2_j = e^{0.2 src_j + p_j} is folded into the
    projection rows (and the ones column -> denominator) on evacuation.
  - the device computes u[j,i] = M_ij * max(d8_i * q8_j, 1) with
    d8 = e^{0.8 dst} (broadcast on-chip from a host row), q8 = e^{0.8 src}
    (per-partition scalar): one tensor_scalar mult per head + one fused
    scalar_tensor_tensor (max 1, mul mask) per head-pair. No activation-
    function table work in the main loop at all.

Schedule: aggregation runs jt-major so it paces with att production
(6 chains live in PSUM during the loop, the last 2 run right after);
head-3's scalar mult rides the otherwise-idle ACT engine.
"""

import numpy as np

import concourse.bass as bass
import concourse.tile as tile
from concourse import bacc, mybir
from concourse.bass_utils import run_bass_kernel_spmd
from concourse.masks import make_identity


def _install_ntff_shim():
    """Provide antenv.axon_hooks if the image lacks it, wiring the NTFF
    profile hook to libaxon_pjrt.so so trace=True runs can report HW time."""
    try:
        import antenv.axon_hooks  # noqa: F401

        return
    except ImportError:
        pass
    try:
        import sys
        import types

        import antenv

        mod = types.ModuleType("antenv.axon_hooks")
        state = {"hook": None}
        mod.set_axon_ntff_profile_hook = lambda h: state.__setitem__("hook", h)
        mod.get_axon_ntff_profile_hook = lambda: state["hook"]
        sys.modules["antenv.axon_hooks"] = mod
        antenv.axon_hooks = mod
        try:
            from trn_agent_boot.trn_boot import _ntff_profile_via_ctypes

            hook = _ntff_profile_via_ctypes("/opt/axon/libaxon_pjrt.so")
            if hook is not None:
                mod.set_axon_ntff_profile_hook(hook)
        except Exception:
            pass
    except Exception:
        pass


_install_ntff_shim()

B, N, IDIM, O, H = 8, 1024, 64, 32, 4
NT = N // 128  # 8 partition tiles
OC = O + 1  # proj columns + ones column (denominator)
WC = H * OC  # 132
F32 = mybir.dt.float32
BF16 = mybir.dt.bfloat16

# aggregation chains: 6 run jt-major inside the loop, 2 right after
WAVE_A = [(0, 0), (0, 1), (1, 0), (1, 1), (2, 0), (3, 0)]
WAVE_B = [(2, 1), (3, 1)]

_NC_CACHE = {}


def _build_nc():
    nc = bacc.Bacc()
    # cst = [xT (row 64 = ones) | wcat]: proj matmul inputs in one DMA
    cst = nc.declare_dram_parameter("cst", [IDIM + 1, N + WC], F32, isOutput=False)
    # per-partition scalars: [:, 0:32] = q8 = e^{0.8 src}, [:, 32:64] = s2
    sc = nc.declare_dram_parameter("sc", [128, 2 * NT * H], F32, isOutput=False)
    # e^{0.8 dst} as a single row; broadcast across partitions on-chip
    d8r = nc.declare_dram_parameter("d8r", [1, H * N], F32, isOutput=False)
    # 0/1 mask, transposed (source j on rows), duplicated per jt: [M | M]
    adjm2 = nc.declare_dram_parameter("adjm2", [128, NT * 2048], BF16, isOutput=False)
    out = nc.declare_dram_parameter("out", [N, H * O], F32, isOutput=True)

    Mul = mybir.AluOpType.mult
    Max = mybir.AluOpType.max
    Copy = mybir.ActivationFunctionType.Copy

    with tile.TileContext(nc) as tc:
        with (
            tc.tile_pool(name="consts", bufs=1) as consts,
            tc.tile_pool(name="gp", bufs=2) as gp,
            tc.tile_pool(name="attp", bufs=8) as attp,
            tc.tile_pool(name="sbp", bufs=8) as sbp,
            tc.tile_pool(name="d4p", bufs=2) as d4p,
            tc.tile_pool(name="pwork", bufs=2, space="PSUM") as pwork,
            tc.tile_pool(name="accp", bufs=6, space="PSUM") as accp,
        ):
            sb_cst = consts.tile([IDIM + 1, N + WC], F32)
            nc.sync.dma_start(out=sb_cst, in_=cst[:, :])
            sb_sc = consts.tile([128, 2 * NT * H], F32)
            nc.sync.dma_start(out=sb_sc, in_=sc[:, :])
            sb_d8r = consts.tile([1, H * N], F32)
            nc.sync.dma_start(out=sb_d8r, in_=d8r[:, :])
            sb_adjm2 = consts.tile([128, NT * 2048], BF16)
            for jt in range(NT):
                nc.sync.dma_start(
                    out=sb_adjm2[:, jt * 2048 : (jt + 1) * 2048],
                    in_=adjm2[:, jt * 2048 : (jt + 1) * 2048],
                )
            sb_d8b = consts.tile([128, H * N], BF16)
            sb_proj = consts.tile([128, NT * WC], BF16)
            out_sb = consts.tile([128, N], F32)
            ones1 = consts.tile([1, 128], F32)
            nc.gpsimd.memset(ones1, 1.0)
            ident = consts.tile([128, 128], F32)
            make_identity(nc, ident)

            # --- broadcast e^{0.8 dst} row across partitions: K=1 outer
            # product on PE, evacuate to bf16 (alternate DVE/ACT) ---
            for q in range(NT):
                pb = pwork.tile([128, 512], F32, tag="pw", name=f"bc{q}")
                nc.tensor.matmul(
                    pb[:, :],
                    lhsT=ones1[:, :],
                    rhs=sb_d8r[:, q * 512 : (q + 1) * 512],
                    start=True,
                    stop=True,
                )
                dst = sb_d8b[:, q * 512 : (q + 1) * 512]
                if q % 2 == 0:
                    nc.scalar.copy(out=dst, in_=pb[:, :])
                else:
                    nc.vector.tensor_copy(out=dst, in_=pb[:, :])

            # --- proj (all heads, one matmul per jt), s2-scaled + cast to
            # bf16 on PSUM->SBUF evacuation (ACT, per-partition scale) ---
            for jt in range(NT):
                pp = pwork.tile([128, 512], F32, tag="pw", name=f"pp{jt}")
                nc.tensor.matmul(
                    pp[:, :WC],
                    lhsT=sb_cst[:, jt * 128 : (jt + 1) * 128],
                    rhs=sb_cst[:, N : N + WC],
                    start=True,
                    stop=True,
                )
                for h in range(H):
                    c = NT * H + jt * H + h
                    nc.scalar.activation(
                        out=sb_proj[:, jt * WC + h * OC : jt * WC + (h + 1) * OC],
                        in_=pp[:, h * OC : (h + 1) * OC],
                        func=Copy,
                        scale=sb_sc[:, c : c + 1],
                    )

            # --- main loop: w = d8_i * q8_j (DVE h0-2, ACT h3), then the
            # fused att = max(w, 1) * M per head-pair; aggregation chains
            # for WAVE_A advance jt-major right behind the att tiles ---
            accs = {
                hh: accp.tile([33, 512], F32, tag="acc", name=f"acc{hh[0]}_{hh[1]}")
                for hh in WAVE_A
            }
            atts = []
            for jt in range(NT):
                g = gp.tile([128, H * N], BF16, tag="g")
                for h in range(3):
                    nc.vector.tensor_scalar_mul(
                        out=g[:, h * N : (h + 1) * N],
                        in0=sb_d8b[:, h * N : (h + 1) * N],
                        scalar1=sb_sc[:, jt * H + h : jt * H + h + 1],
                    )
                nc.scalar.activation(
                    out=g[:, 3 * N : 4 * N],
                    in_=sb_d8b[:, 3 * N : 4 * N],
                    func=Copy,
                    scale=sb_sc[:, jt * H + 3 : jt * H + 4],
                )
                att = attp.tile([128, H * N], BF16, tag="att", name=f"att_{jt}")
                for pr in range(2):
                    nc.vector.scalar_tensor_tensor(
                        out=att[:, pr * 2048 : (pr + 1) * 2048],
                        in0=g[:, pr * 2048 : (pr + 1) * 2048],
                        scalar=1.0,
                        in1=sb_adjm2[:, jt * 2048 : (jt + 1) * 2048],
                        op0=Max,
                        op1=Mul,
                    )
                atts.append(att)
                for h, half in WAVE_A:
                    nc.tensor.matmul(
                        accs[(h, half)][:, :],
                        lhsT=sb_proj[:, jt * WC + h * OC : jt * WC + (h + 1) * OC],
                        rhs=att[:, h * N + half * 512 : h * N + half * 512 + 512],
                        start=(jt == 0),
                        stop=(jt == NT - 1),
                    )

            sbaccs = {}
            for hh in WAVE_A:
                sba = sbp.tile([33, 512], F32, tag="sba", name=f"sba{hh[0]}_{hh[1]}")
                nc.scalar.copy(out=sba, in_=accs[hh][:, :])
                sbaccs[hh] = sba

            def finalize(it):
                half, q = it // 4, it % 4
                tp = pwork.tile([128, 512], F32, tag="pw", name=f"tp{it}")
                for h in range(H):
                    nc.tensor.transpose(
                        tp[:, h * OC : (h + 1) * OC],
                        sbaccs[(h, half)][:, q * 128 : (q + 1) * 128],
                        ident[:33, :33],
                    )
                d4 = d4p.tile([128, H], F32, tag="d4")
                nc.vector.reciprocal(out=d4, in_=tp[:, O : WC : OC])
                for h in range(H):
                    ob = out_sb[:, it * 128 + h * O : it * 128 + (h + 1) * O]
                    if h % 2 == 0:
                        nc.vector.tensor_scalar_mul(
                            out=ob,
                            in0=tp[:, h * OC : h * OC + O],
                            scalar1=d4[:, h : h + 1],
                        )
                    else:
                        nc.scalar.activation(
                            out=ob,
                            in_=tp[:, h * OC : h * OC + O],
                            func=Copy,
                            scale=d4[:, h : h + 1],
                        )
                nc.sync.dma_start(
                    out=out[it * 128 : (it + 1) * 128, :],
                    in_=out_sb[:, it * 128 : (it + 1) * 128],
                )

            # half-0 row tiles finalize while WAVE_B aggregates
            for it in range(4):
                finalize(it)

            waccs = {
                hh: accp.tile([33, 512], F32, tag="acc", name=f"acc{hh[0]}_{hh[1]}")
                for hh in WAVE_B
            }
            for jt in range(NT):
                for h, half in WAVE_B:
                    nc.tensor.matmul(
                        waccs[(h, half)][:, :],
                        lhsT=sb_proj[:, jt * WC + h * OC : jt * WC + (h + 1) * OC],
                        rhs=atts[jt][:, h * N + half * 512 : h * N + half * 512 + 512],
                        start=(jt == 0),
                        stop=(jt == NT - 1),
                    )
            for hh in WAVE_B:
                sba = sbp.tile([33, 512], F32, tag="sba", name=f"sba{hh[0]}_{hh[1]}")
                nc.scalar.copy(out=sba, in_=waccs[hh][:, :])
                sbaccs[hh] = sba
            for it in range(4, NT):
                finalize(it)
    nc.finalize()
    return nc


def _get_nc():
    if "nc" not in _NC_CACHE:
        _NC_CACHE["nc"] = _build_nc()
    return _NC_CACHE["nc"]


def _prep_inputs(x, adj, source_prior, beta, weight, attn_src, attn_dst, bias):
    import ml_dtypes

    x = np.asarray(x, np.float32)
    adj = np.asarray(adj)
    source_prior = np.asarray(source_prior, np.float32)
    beta = np.asarray(beta, np.float32)
    weight = np.asarray(weight, np.float32)
    attn_src = np.asarray(attn_src, np.float32)
    attn_dst = np.asarray(attn_dst, np.float32)
    bias = np.asarray(bias, np.float32)

    # [N j, N i] 0/1 mask (j = source on rows), duplicated per jt: [M | M]
    m01 = (adj.T != 0).astype(ml_dtypes.bfloat16)
    adjm2 = np.empty((128, NT * 2048), ml_dtypes.bfloat16)
    for jt in range(NT):
        blk = m01[jt * 128 : (jt + 1) * 128, :]
        adjm2[:, jt * 2048 : jt * 2048 + N] = blk
        adjm2[:, jt * 2048 + N : (jt + 1) * 2048] = blk
    adjm2 = np.ascontiguousarray(adjm2)

    wcat = np.zeros((IDIM + 1, WC), np.float32)
    for h in range(H):
        wcat[:IDIM, h * OC : h * OC + O] = weight[h]
        wcat[IDIM, h * OC : h * OC + O] = bias[h]
        wcat[IDIM, h * OC + O] = 1.0  # ones column -> softmax denominator

    gain = np.logaddexp(0.0, beta).astype(np.float32)  # softplus

    in_maps = []
    for b in range(B):
        # host-side scores (cheap, O(N*H*I))
        proj = np.einsum("ni,hio->hno", x[b], weight) + bias[:, None, :]  # [H,N,O]
        src = np.einsum("hno,ho->hn", proj, attn_src)  # [H,N]
        dst = np.einsum("hno,ho->hn", proj, attn_dst)  # [H,N]
        p = gain[:, None] * source_prior[b][None, :]  # [H,N]
        s2 = np.exp(0.2 * src + p).astype(np.float32)  # [H,N]
        q8 = np.exp(0.8 * src).astype(np.float32)
        d8 = np.exp(0.8 * dst).astype(np.float32)

        xT = np.ones((IDIM + 1, N), np.float32)
        xT[:IDIM] = x[b].T
        cst = np.ascontiguousarray(np.concatenate([xT, wcat], axis=1))

        d8row = np.ascontiguousarray(d8.reshape(1, H * N))

        # sc[:, jt*H+h] = q8[h, jt*128+jj]; sc[:, 32 + jt*H+h] = s2[...]
        sccols = np.empty((128, 2 * NT * H), np.float32)
        q8t = q8.T.reshape(NT, 128, H).transpose(1, 0, 2).reshape(128, NT * H)
        s2t = s2.T.reshape(NT, 128, H).transpose(1, 0, 2).reshape(128, NT * H)
        sccols[:, : NT * H] = q8t
        sccols[:, NT * H :] = s2t
        sccols = np.ascontiguousarray(sccols)

        in_maps.append({"cst": cst, "sc": sccols, "d8r": d8row, "adjm2": adjm2})
    return in_maps


def _run(inputs, trace=False):
    in_maps = _prep_inputs(**inputs)
    nc = _get_nc()
    res = run_bass_kernel_spmd(nc, in_maps, list(range(B)), trace=trace)
    out = np.stack([res.results[b]["out"] for b in range(B)]).astype(np.float32)
    return out, res


def kernel(**inputs):
    out, _ = _run(inputs, trace=False)
    return out


# revision 16
# speedup vs baseline: 1.3798x; 1.3798x over previous
"""Dense graph-attention layer (GAT) on 8 Trainium2 NeuronCores.

Sharding: data-parallel over batch B=8 -> one batch element per core.
Adjacency mask and per-head params are replicated.

Key identity (exact, no approximation):
  leaky_relu(t, 0.2) = 0.2*t + 0.8*relu(t)
  exp(leaky(dst_i + src_j) + p_j)
    = e^{0.2 dst_i} * e^{0.2 src_j + p_j} * max(e^{0.8(dst_i+src_j)}, 1)

  - the per-i factor e^{0.2 dst_i} cancels between softmax numerator and
    denominator -> dropped entirely.
  - the per-j factor s2_j = e^{0.2 src_j + p_j} is folded into the
    projection rows (and the ones column -> denominator) on evacuation.
  - the device computes u[j,i] = M_ij * max(d8_i * q8_j, 1) with
    d8 = e^{0.8 dst} (broadcast on-chip from a host row), q8 = e^{0.8 src}
    (per-partition scalar): one tensor_scalar mult per head + one fused
    scalar_tensor_tensor (max 1, mul mask) per head-pair. No activation-
    function table work in the main loop at all.

Schedule: aggregation runs jt-major so it paces with att production
(6 chains live in PSUM during the loop, the last 2 run right after);
head-3's scalar mult rides the otherwise-idle ACT engine.
"""

import numpy as np

import concourse.bass as bass
import concourse.tile as tile
from concourse import bacc, mybir
from concourse.bass_utils import run_bass_kernel_spmd
from concourse.masks import make_identity


def _install_ntff_shim():
    """Provide antenv.axon_hooks if the image lacks it, wiring the NTFF
    profile hook to libaxon_pjrt.so so trace=True runs can report HW time."""
    try:
        import antenv.axon_hooks  # noqa: F401

        return
    except ImportError:
        pass
    try:
        import sys
        import types

        import antenv

        mod = types.ModuleType("antenv.axon_hooks")
        state = {"hook": None}
        mod.set_axon_ntff_profile_hook = lambda h: state.__setitem__("hook", h)
        mod.get_axon_ntff_profile_hook = lambda: state["hook"]
        sys.modules["antenv.axon_hooks"] = mod
        antenv.axon_hooks = mod
        try:
            from trn_agent_boot.trn_boot import _ntff_profile_via_ctypes

            hook = _ntff_profile_via_ctypes("/opt/axon/libaxon_pjrt.so")
            if hook is not None:
                mod.set_axon_ntff_profile_hook(hook)
        except Exception:
            pass
    except Exception:
        pass


_install_ntff_shim()

B, N, IDIM, O, H = 8, 1024, 64, 32, 4
NT = N // 128  # 8 partition tiles
OC = O + 1  # proj columns + ones column (denominator)
WC = H * OC  # 132
F32 = mybir.dt.float32
BF16 = mybir.dt.bfloat16

# aggregation chains: 6 run jt-major inside the loop, 2 right after
WAVE_A = [(0, 0), (0, 1), (1, 0), (1, 1), (2, 0), (3, 0)]
WAVE_B = [(2, 1), (3, 1)]

_NC_CACHE = {}


def _build_nc():
    nc = bacc.Bacc()
    # cst = [xT (row 64 = ones) | wcat]: proj matmul inputs in one DMA
    cst = nc.declare_dram_parameter("cst", [IDIM + 1, N + WC], F32, isOutput=False)
    # per-partition scalars: [:, 0:32] = q8 = e^{0.8 src}, [:, 32:64] = s2
    sc = nc.declare_dram_parameter("sc", [128, 2 * NT * H], F32, isOutput=False)
    # broadcast e^{0.8 dst} rows (replicated on host), [h*N + i] column order
    d8b = nc.declare_dram_parameter("d8b", [128, H * N], BF16, isOutput=False)
    # 0/1 mask, transposed (source j on rows), duplicated per jt: [M | M]
    adjm2 = nc.declare_dram_parameter("adjm2", [128, NT * 2048], BF16, isOutput=False)
    out = nc.declare_dram_parameter("out", [N, H * O], F32, isOutput=True)

    Mul = mybir.AluOpType.mult
    Max = mybir.AluOpType.max
    Copy = mybir.ActivationFunctionType.Copy

    with tile.TileContext(nc) as tc:
        with (
            tc.tile_pool(name="consts", bufs=1) as consts,
            tc.tile_pool(name="gp", bufs=2) as gp,
            tc.tile_pool(name="attp", bufs=8) as attp,
            tc.tile_pool(name="sbp", bufs=8) as sbp,
            tc.tile_pool(name="d4p", bufs=2) as d4p,
            tc.tile_pool(name="pwork", bufs=2, space="PSUM") as pwork,
            tc.tile_pool(name="accp", bufs=6, space="PSUM") as accp,
        ):
            sb_cst = consts.tile([IDIM + 1, N + WC], F32)
            nc.sync.dma_start(out=sb_cst, in_=cst[:, :])
            sb_sc = consts.tile([128, 2 * NT * H], F32)
            nc.sync.dma_start(out=sb_sc, in_=sc[:, :])
            sb_d8b = consts.tile([128, H * N], BF16)
            nc.sync.dma_start(out=sb_d8b[:, : 2 * N], in_=d8b[:, : 2 * N])
            sb_adjm2 = consts.tile([128, NT * 2048], BF16)
            nc.sync.dma_start(out=sb_adjm2[:, :2048], in_=adjm2[:, :2048])
            nc.sync.dma_start(out=sb_d8b[:, 2 * N :], in_=d8b[:, 2 * N :])
            for jt in range(1, NT):
                nc.sync.dma_start(
                    out=sb_adjm2[:, jt * 2048 : (jt + 1) * 2048],
                    in_=adjm2[:, jt * 2048 : (jt + 1) * 2048],
                )
            sb_proj = consts.tile([128, NT * WC], BF16)
            out_sb = consts.tile([128, N], F32)
            ident = consts.tile([128, 128], F32)
            make_identity(nc, ident)

            # --- proj (all heads, one matmul per jt), s2-scaled + cast to
            # bf16 on PSUM->SBUF evacuation (ACT, per-partition scale) ---
            for jt in range(NT):
                pp = pwork.tile([128, 512], F32, tag="pw", name=f"pp{jt}")
                nc.tensor.matmul(
                    pp[:, :WC],
                    lhsT=sb_cst[:, jt * 128 : (jt + 1) * 128],
                    rhs=sb_cst[:, N : N + WC],
                    start=True,
                    stop=True,
                )
                for h in range(H):
                    c = NT * H + jt * H + h
                    nc.scalar.activation(
                        out=sb_proj[:, jt * WC + h * OC : jt * WC + (h + 1) * OC],
                        in_=pp[:, h * OC : (h + 1) * OC],
                        func=Copy,
                        scale=sb_sc[:, c : c + 1],
                    )

            # --- main loop: g = max(d8_i * q8_j, 1) (dual-op TS), then
            # att = g * M per head-pair; aggregation chains for WAVE_A
            # advance jt-major right behind the att tiles ---
            accs = {
                hh: accp.tile([33, 512], F32, tag="acc", name=f"acc{hh[0]}_{hh[1]}")
                for hh in WAVE_A
            }
            atts = []
            for jt in range(NT):
                g = gp.tile([128, H * N], BF16, tag="g")
                for h in range(H):
                    nc.vector.tensor_scalar(
                        out=g[:, h * N : (h + 1) * N],
                        in0=sb_d8b[:, h * N : (h + 1) * N],
                        scalar1=sb_sc[:, jt * H + h : jt * H + h + 1],
                        scalar2=1.0,
                        op0=Mul,
                        op1=Max,
                    )
                att = attp.tile([128, H * N], BF16, tag="att", name=f"att_{jt}")
                for pr in range(2):
                    nc.vector.tensor_mul(
                        out=att[:, pr * 2048 : (pr + 1) * 2048],
                        in0=g[:, pr * 2048 : (pr + 1) * 2048],
                        in1=sb_adjm2[:, jt * 2048 : (jt + 1) * 2048],
                    )
                atts.append(att)
                for h, half in WAVE_A:
                    nc.tensor.matmul(
                        accs[(h, half)][:, :],
                        lhsT=sb_proj[:, jt * WC + h * OC : jt * WC + (h + 1) * OC],
                        rhs=att[:, h * N + half * 512 : h * N + half * 512 + 512],
                        start=(jt == 0),
                        stop=(jt == NT - 1),
                    )

            sbaccs = {}
            for hh in WAVE_A:
                sba = sbp.tile([33, 512], F32, tag="sba", name=f"sba{hh[0]}_{hh[1]}")
                nc.scalar.copy(out=sba, in_=accs[hh][:, :])
                sbaccs[hh] = sba

            def finalize(it):
                half, q = it // 4, it % 4
                tp = pwork.tile([128, 512], F32, tag="pw", name=f"tp{it}")
                for h in range(H):
                    nc.tensor.transpose(
                        tp[:, h * OC : (h + 1) * OC],
                        sbaccs[(h, half)][:, q * 128 : (q + 1) * 128],
                        ident[:33, :33],
                    )
                d4 = d4p.tile([128, H], F32, tag="d4")
                nc.vector.reciprocal(out=d4, in_=tp[:, O : WC : OC])
                for h in range(H):
                    ob = out_sb[:, it * 128 + h * O : it * 128 + (h + 1) * O]
                    if h % 2 == 0:
                        nc.vector.tensor_scalar_mul(
                            out=ob,
                            in0=tp[:, h * OC : h * OC + O],
                            scalar1=d4[:, h : h + 1],
                        )
                    else:
                        nc.scalar.activation(
                            out=ob,
                            in_=tp[:, h * OC : h * OC + O],
                            func=Copy,
                            scale=d4[:, h : h + 1],
                        )
                nc.sync.dma_start(
                    out=out[it * 128 : (it + 1) * 128, :],
                    in_=out_sb[:, it * 128 : (it + 1) * 128],
                )

            # half-0 row tiles finalize while WAVE_B aggregates
            for it in range(4):
                finalize(it)

            waccs = {
                hh: accp.tile([33, 512], F32, tag="acc", name=f"acc{hh[0]}_{hh[1]}")
                for hh in WAVE_B
            }
            for jt in range(NT):
                for h, half in WAVE_B:
                    nc.tensor.matmul(
                        waccs[(h, half)][:, :],
                        lhsT=sb_proj[:, jt * WC + h * OC : jt * WC + (h + 1) * OC],
                        rhs=atts[jt][:, h * N + half * 512 : h * N + half * 512 + 512],
                        start=(jt == 0),
                        stop=(jt == NT - 1),
                    )
            for hh in WAVE_B:
                sba = sbp.tile([33, 512], F32, tag="sba", name=f"sba{hh[0]}_{hh[1]}")
                nc.scalar.copy(out=sba, in_=waccs[hh][:, :])
                sbaccs[hh] = sba
            for it in range(4, NT):
                finalize(it)
    nc.finalize()
    return nc


def _get_nc():
    if "nc" not in _NC_CACHE:
        _NC_CACHE["nc"] = _build_nc()
    return _NC_CACHE["nc"]


def _prep_inputs(x, adj, source_prior, beta, weight, attn_src, attn_dst, bias):
    import ml_dtypes

    x = np.asarray(x, np.float32)
    adj = np.asarray(adj)
    source_prior = np.asarray(source_prior, np.float32)
    beta = np.asarray(beta, np.float32)
    weight = np.asarray(weight, np.float32)
    attn_src = np.asarray(attn_src, np.float32)
    attn_dst = np.asarray(attn_dst, np.float32)
    bias = np.asarray(bias, np.float32)

    # [N j, N i] 0/1 mask (j = source on rows), duplicated per jt: [M | M]
    m01 = (adj.T != 0).astype(ml_dtypes.bfloat16)
    adjm2 = np.empty((128, NT * 2048), ml_dtypes.bfloat16)
    for jt in range(NT):
        blk = m01[jt * 128 : (jt + 1) * 128, :]
        adjm2[:, jt * 2048 : jt * 2048 + N] = blk
        adjm2[:, jt * 2048 + N : (jt + 1) * 2048] = blk
    adjm2 = np.ascontiguousarray(adjm2)

    wcat = np.zeros((IDIM + 1, WC), np.float32)
    for h in range(H):
        wcat[:IDIM, h * OC : h * OC + O] = weight[h]
        wcat[IDIM, h * OC : h * OC + O] = bias[h]
        wcat[IDIM, h * OC + O] = 1.0  # ones column -> softmax denominator

    gain = np.logaddexp(0.0, beta).astype(np.float32)  # softplus

    in_maps = []
    for b in range(B):
        # host-side scores (cheap, O(N*H*I))
        proj = np.einsum("ni,hio->hno", x[b], weight) + bias[:, None, :]  # [H,N,O]
        src = np.einsum("hno,ho->hn", proj, attn_src)  # [H,N]
        dst = np.einsum("hno,ho->hn", proj, attn_dst)  # [H,N]
        p = gain[:, None] * source_prior[b][None, :]  # [H,N]
        s2 = np.exp(0.2 * src + p).astype(np.float32)  # [H,N]
        q8 = np.exp(0.8 * src).astype(np.float32)
        d8 = np.exp(0.8 * dst).astype(np.float32)

        xT = np.ones((IDIM + 1, N), np.float32)
        xT[:IDIM] = x[b].T
        cst = np.ascontiguousarray(np.concatenate([xT, wcat], axis=1))

        d8row = d8.reshape(H * N).astype(ml_dtypes.bfloat16)
        d8brep = np.ascontiguousarray(np.broadcast_to(d8row[None, :], (128, H * N)))

        # sc[:, jt*H+h] = q8[h, jt*128+jj]; sc[:, 32 + jt*H+h] = s2[...]
        sccols = np.empty((128, 2 * NT * H), np.float32)
        q8t = q8.T.reshape(NT, 128, H).transpose(1, 0, 2).reshape(128, NT * H)
        s2t = s2.T.reshape(NT, 128, H).transpose(1, 0, 2).reshape(128, NT * H)
        sccols[:, : NT * H] = q8t
        sccols[:, NT * H :] = s2t
        sccols = np.ascontiguousarray(sccols)

        in_maps.append({"cst": cst, "sc": sccols, "d8b": d8brep, "adjm2": adjm2})
    return in_maps


def _run(inputs, trace=False):
    in_maps = _prep_inputs(**inputs)
    nc = _get_nc()
    res = run_bass_kernel_spmd(nc, in_maps, list(range(B)), trace=trace)
    out = np.stack([res.results[b]["out"] for b in range(B)]).astype(np.float32)
    return out, res


def kernel(**inputs):
    out, _ = _run(inputs, trace=False)
    return out
